# revision 25
# baseline (speedup 1.0000x reference)
import numpy as np

# nn_Attention: B=256, N=65, DIM=1024, HEADS=16, DH=64 across 8 cores (32 batches/core)
B, N, DIM, HEADS, DH = 256, 65, 1024, 16, 64
NCORES = 8
BPC = B // NCORES            # 32 batches per core
TOK = BPC * N                # 2080 tokens per core
BB = 4                       # batches per block
NBLK = BPC // BB             # 8 blocks
TB = BB * N                  # 260 tokens per block
BN_EPS = 1e-5


def _build(nc_mod, mybir, bass):
    f32 = mybir.dt.float32
    bf16 = mybir.dt.bfloat16
    Alu = mybir.AluOpType
    Act = mybir.ActivationFunctionType
    from concourse.tile import TileContext

    nc = nc_mod
    xt = nc.declare_dram_parameter("xt", [DIM, TOK], bf16, isOutput=False)
    wqkvt = nc.declare_dram_parameter("wqkvt", [DIM, 3 * DIM], bf16, isOutput=False)
    woutt = nc.declare_dram_parameter("woutt", [DIM, DIM], bf16, isOutput=False)
    wconvt = nc.declare_dram_parameter("wconvt", [9, N, 68], bf16, isOutput=False)
    st = nc.declare_dram_parameter("st", [N, 2], f32, isOutput=False)
    bqkvc = nc.declare_dram_parameter("bqkvc", [128, 16], f32, isOutput=False)
    ident = nc.declare_dram_parameter("ident", [128, 128], bf16, isOutput=False)
    out = nc.declare_dram_parameter("out", [DIM, TOK], f32, isOutput=True)

    from contextlib import ExitStack
    with TileContext(nc) as tc:
        with ExitStack() as es:
            P = lambda *a, **k: es.enter_context(tc.tile_pool(*a, **k))
            cp = P(name="consts", bufs=1)
            xtp = P(name="xtp", bufs=2)
            qtp = P(name="qtp", bufs=2)
            ktp = P(name="ktp", bufs=2)
            vtp = P(name="vtp", bufs=2)
            vbp = P(name="vbp", bufs=2)
            expp = P(name="exps", bufs=4)
            recp = P(name="recips", bufs=4)
            resp = P(name="resp", bufs=2)
            bnp = P(name="bnp", bufs=2)
            rtp = P(name="rtp", bufs=2)
            osbp = P(name="osb", bufs=2)
            pa = P(name="pa", bufs=2, space="PSUM")     # [128,260] f32: QKV evac + outproj
            psml = P(name="psml", bufs=2, space="PSUM")  # [65/128,<=260]: dots4 + transposes
            pat = P(name="pat", bufs=2, space="PSUM")   # [65,260] f32: attnV 4-head groups
            pcv = P(name="pconv", bufs=2, space="PSUM")  # [68,512] f32: conv halves

            # ---- resident constants ----
            wqkv_sb = cp.tile([128, 8 * 3 * DIM], bf16, tag="wqkv")
            nc.sync.dma_start(
                out=wqkv_sb[:].rearrange("p (a n) -> p a n", a=8),
                in_=wqkvt[:].rearrange("(a p) n -> p a n", p=128),
            )
            wqkv = wqkv_sb[:].rearrange("p (a n) -> p a n", a=8)

            wout_sb = cp.tile([128, 8 * DIM], bf16, tag="wout")
            nc.sync.dma_start(
                out=wout_sb[:].rearrange("p (a n) -> p a n", a=8),
                in_=woutt[:].rearrange("(a p) n -> p a n", p=128),
            )
            wout = wout_sb[:].rearrange("p (a n) -> p a n", a=8)

            wconv_sb = cp.tile([N, 9 * 68], bf16, tag="wconv")
            nc.sync.dma_start(
                out=wconv_sb[:].rearrange("i (t o) -> i t o", t=9),
                in_=wconvt[:].rearrange("t i o -> i t o"),
            )
            wconv = wconv_sb[:].rearrange("i (t o) -> i t o", t=9)

            st_sb = cp.tile([N, 2], f32, tag="st")
            nc.sync.dma_start(out=st_sb[:], in_=st[:])
            bq_sb = cp.tile([128, 16], f32, tag="bq")
            nc.sync.dma_start(out=bq_sb[:], in_=bqkvc[:])
            id_sb = cp.tile([128, 128], bf16, tag="id")
            nc.sync.dma_start(out=id_sb[:], in_=ident[:])

            SCALE = float(DIM) ** -0.5

            def dma_x(blk):
                t0 = blk * TB
                xt_sb = xtp.tile([128, 8 * TB], bf16, tag="xt")
                xtv = xt_sb[:].rearrange("p (a n) -> p a n", a=8)
                nc.sync.dma_start(
                    out=xtv,
                    in_=xt[:].rearrange("(a p) n -> p a n", p=128)[:, :, t0:t0 + TB],
                )
                return xtv

            def alloc_qkv():
                qt_sb = qtp.tile([128, 8 * TB], bf16, tag="qt")
                qtv = qt_sb[:].rearrange("p (a n) -> p a n", a=8)
                kt_sb = ktp.tile([128, 8 * TB], bf16, tag="kt")
                ktv = kt_sb[:].rearrange("p (a n) -> p a n", a=8)
                vt_sb = vtp.tile([128, 8 * TB], bf16, tag="vt")
                vtv = vt_sb[:].rearrange("p (a n) -> p a n", a=8)
                return qtv, ktv, vtv

            def proj_group(xtv, dsts, gi):
                # gi in 0..23: one [128,260] output tile of the QKV projection
                (qtv, ktv, vtv) = dsts
                which, m = divmod(gi, 8)
                dst, coff, boff = ((qtv, 0, 0), (ktv, DIM, 8),
                                   (vtv, 2 * DIM, None))[which]
                pqk = pa.tile([128, TB], f32, tag="pA")
                for ki in range(8):
                    nc.tensor.matmul(
                        pqk[:],
                        wqkv[:, ki, coff + m * 128:coff + (m + 1) * 128],
                        xtv[:, ki, :],
                        start=(ki == 0), stop=(ki == 7),
                    )
                if boff is not None:
                    nc.vector.tensor_scalar_add(
                        dst[:, m, :], pqk[:], bq_sb[:, boff + m:boff + m + 1]
                    )
                else:
                    nc.scalar.activation(dst[:, m, :], pqk[:], Act.Copy)

            # software pipeline: block 0 projected in a dense prologue; while
            # block n's attention runs, block n+1's projection groups are
            # sprinkled between sub-phases to keep the PE densely busy (the
            # HAM clock-gate re-throttles to 1.2 GHz when the PE micro-idles)
            xtv_cur = dma_x(0)
            dsts_cur = alloc_qkv()
            for gi in list(range(16, 24)) + list(range(16)):
                proj_group(xtv_cur, dsts_cur, gi)

            def out_group(rtv, t0, m, f0, fw):
                po2 = pa.tile([128, TB], f32, tag="pA")
                for ki in range(8):
                    nc.tensor.matmul(
                        po2[:, 0:fw],
                        wout[:, ki, m * 128:(m + 1) * 128],
                        rtv[:, ki, f0:f0 + fw],
                        start=(ki == 0), stop=(ki == 7),
                    )
                ob = osbp.tile([128, TB], f32, tag="ob")
                nc.vector.tensor_copy(ob[:, 0:fw], po2[:, 0:fw])
                nc.sync.dma_start(
                    out=out[m * 128:(m + 1) * 128, t0 + f0:t0 + f0 + fw],
                    in_=ob[:, 0:fw],
                )

            oproj_pending = []
            oproj_done = False
            for blk in range(NBLK):
                t0 = blk * TB
                qtv, ktv, vtv = dsts_cur
                pending = []
                if blk + 1 < NBLK:
                    xtv_nxt = dma_x(blk + 1)
                    dsts_nxt = alloc_qkv()
                    pending = [
                        (lambda g=gi: proj_group(xtv_nxt, dsts_nxt, g))
                        for gi in (list(range(16, 24)) + list(range(16)))
                    ]

                def filler(k=1):
                    for _ in range(k):
                        if pending:
                            pending.pop(0)()

                rt = rtp.tile([128, 8 * TB], bf16, tag="rt")
                rtv = rt[:].rearrange("p (a n) -> p a n", a=8)

                for bi in range(BB):
                    toff = bi * N
                    # ---- per-batch v tile [65, 1+16*66(+1 slack)] ----
                    # col 0 zero; head h at 1+66h: v(64), ones, zero.
                    # attention reads [v|ones]; conv dx=-1 reads the
                    # preceding zero col; dx=+1 spuriously reads the ones
                    # col into out x=63 (corrected host-side).
                    vb = vbp.tile([N, 1 + HEADS * 66 + 1], bf16, tag="vb")
                    vbh = vb[:, 1:1 + HEADS * 66].rearrange("p (h c) -> p h c", c=66)
                    nc.gpsimd.memset(
                        vb[:, 0:HEADS * 66].rearrange(
                            "p (h c) -> p h c", c=66)[:, :, 0:1], 0.0)
                    nc.gpsimd.memset(vbh[:, :, DH:DH + 1], 1.0)
                    nc.gpsimd.memset(vbh[:, HEADS - 1:HEADS, DH + 1:DH + 2], 0.0)
                    for a in range(8):
                        pt2 = psml.tile([N, 260], bf16, tag="pS")
                        nc.tensor.transpose(
                            pt2[:, 0:128], vtv[:, a, toff:toff + N], id_sb[:])
                        if a % 2 == 0:
                            nc.vector.tensor_copy(
                                vbh[:, a * 2:(a + 1) * 2, 0:DH],
                                pt2[:, 0:128].rearrange("p (h c) -> p h c", c=DH))
                        else:
                            nc.scalar.activation(
                                vbh[:, a * 2:(a + 1) * 2, 0:DH],
                                pt2[:, 0:128].rearrange("p (h c) -> p h c", c=DH),
                                Act.Copy)

                    res = resp.tile([N, DIM], bf16, tag="res")
                    bn = bnp.tile([N, DIM], bf16, tag="bn")

                    # attention in 4-head groups, conv halves interleaved to
                    # keep the PE busy while scalar exp runs.
                    # heads grouped by parity: alternating the stationary
                    # base-partition (0/64) between back-to-back matmuls into
                    # one psum tile faults on HW, so keep poff constant
                    # within each group.
                    HG = ([0, 2, 4, 6], [1, 3, 5, 7],
                          [8, 10, 12, 14], [9, 11, 13, 15])
                    pd_tiles = {}
                    ex_tiles = {}

                    def dots_pair(half):
                        # even heads use PE rows 0-63, odd rows 64-127, into
                        # DIFFERENT psum banks -> pairs run concurrently
                        pdE = psml.tile([N, 260], f32, tag="pS")
                        pdO = psml.tile([N, 260], f32, tag="pS")
                        for j in range(4):
                            aa = half * 4 + j
                            nc.tensor.matmul(
                                pdE[:, j * N:(j + 1) * N],
                                ktv[0:64, aa, toff:toff + N],
                                qtv[0:64, aa, toff:toff + N],
                                start=True, stop=True,
                            )
                            nc.tensor.matmul(
                                pdO[:, j * N:(j + 1) * N],
                                ktv[64:128, aa, toff:toff + N],
                                qtv[64:128, aa, toff:toff + N],
                                start=True, stop=True,
                            )
                        pd_tiles[2 * half] = pdE
                        pd_tiles[2 * half + 1] = pdO

                    def exp4(g):
                        ex4 = expp.tile([N, 260], bf16, tag="ex")
                        nc.scalar.activation(ex4[:], pd_tiles[g][:], Act.Exp, scale=SCALE)
                        ex_tiles[g] = ex4

                    def attn4(g):
                        po4 = pat.tile([N, 260], f32, tag="po")
                        ex4 = ex_tiles[g]
                        for j, h in enumerate(HG[g]):
                            nc.tensor.matmul(
                                po4[:, j * N:(j + 1) * N],
                                ex4[:, j * N:(j + 1) * N],
                                vbh[:, h, 0:DH + 1],
                                start=True, stop=True,
                            )
                        pov = po4[:].rearrange("p (j c) -> p j c", c=N)
                        rc4 = recp.tile([N, 4], f32, tag="rc")
                        nc.vector.reciprocal(rc4[:], pov[:, :, DH])
                        par, jb = g % 2, (g // 2) * 4
                        nc.vector.tensor_tensor(
                            res[:].rearrange("p (j q c) -> p j q c", q=2, c=DH)[
                                :, jb:jb + 4, par:par + 1, :],
                            po4[:].rearrange("p (j o c) -> p j o c", o=1, c=N)[
                                :, :, :, 0:DH],
                            rc4[:].rearrange("p (j o q) -> p j o q", o=1, q=1
                                             ).to_broadcast([N, 4, 1, DH]),
                            Alu.mult,
                        )

                    conv_state = {}

                    def conv_part(half, taps, start, stop):
                        if start:
                            pc_new = pcv.tile([68, 512], f32, tag="pc")
                            conv_state[half] = pc_new
                        pc = conv_state[half]
                        y0h, y1h = half * 8, half * 8 + 8
                        pcy = pc[:].rearrange("p (y x) -> p y x", x=64)
                        for ti, t in enumerate(taps):
                            dy, dx = t // 3 - 1, t % 3 - 1
                            oy0, oy1 = max(y0h, -dy), min(y1h, 16 - dy)
                            ny = oy1 - oy0
                            off = 1 + dx + 66 * (oy0 + dy)
                            mov = vb[:, off:off + 66 * ny].rearrange(
                                "p (y c) -> p y c", c=66)[:, :, 0:DH]
                            nc.tensor.matmul(
                                pcy[:, oy0 - y0h:oy1 - y0h, :],
                                wconv[:, t, :],
                                mov,
                                start=(start and ti == 0),
                                stop=(stop and ti == len(taps) - 1),
                                skip_group_check=True,
                            )
                        if stop:
                            nc.vector.tensor_scalar(
                                bn[:, half * 512:half * 512 + 512], pc[0:N, :],
                                st_sb[:, 0:1], 0.0, Alu.mult, Alu.add,
                            )

                    def res_t(c8):
                        pt = psml.tile([128, 260], bf16, tag="pS")
                        nc.tensor.transpose(
                            pt[:, 0:N], res[:, c8 * 128:(c8 + 1) * 128], id_sb[:N, :N]
                        )
                        if c8 % 2 == 0:
                            nc.vector.tensor_copy(rtv[:, c8, toff:toff + N], pt[:, 0:N])
                        else:
                            nc.scalar.activation(rtv[:, c8, toff:toff + N], pt[:, 0:N],
                                                 Act.Copy)

                    filler()
                    dots_pair(0)
                    exp4(0)
                    exp4(1)
                    filler()
                    dots_pair(1)
                    exp4(2)
                    exp4(3)
                    conv_part(0, (4, 0, 1, 2, 3), True, False)
                    attn4(0)
                    filler()
                    conv_part(0, (5, 6, 7, 8), False, True)
                    attn4(1)
                    # res half 0 (features 0..511 = heads 0..7) complete after
                    # tt(0), tt(1), BN0 -> add on idle gpsimd, then transposes
                    nc.gpsimd.tensor_add(res[:, 0:512], res[:, 0:512], bn[:, 0:512])
                    filler()
                    conv_part(1, (4, 0, 1, 2, 3), True, False)
                    attn4(2)
                    for c8 in range(4):
                        res_t(c8)
                    filler()
                    conv_part(1, (5, 6, 7, 8), False, True)
                    attn4(3)
                    nc.gpsimd.tensor_add(res[:, 512:1024], res[:, 512:1024],
                                         bn[:, 512:1024])
                    for c8 in range(4, 8):
                        res_t(c8)
                    filler()
                    if blk == NBLK - 1 and bi == 1:
                        pending.extend(
                            lambda m=m: out_group(rtv, t0, m, 0, 2 * N)
                            for m in range(8))
                    if blk == NBLK - 1 and bi == 3:
                        for m in range(8):
                            out_group(rtv, t0, m, 2 * N, 2 * N)
                        oproj_done = True

                # ---- flush remaining next-block projections ----
                filler(len(pending))
                if blk + 1 < NBLK:
                    xtv_cur, dsts_cur = xtv_nxt, dsts_nxt

                # ---- final projection, transposed out ----
                for m, f0, fw in oproj_pending:
                    out_group(rtv, t0, m, f0, fw)
                oproj_pending = []
                for m in range(8):
                    if oproj_done and blk == NBLK - 1:
                        break
                    out_group(rtv, t0, m, 0, TB)
    return nc


def kernel(x, w_qkv, b_qkv, w_out, b_out, conv_w, conv_b,
           bn_gamma, bn_beta, bn_mean, bn_var):
    import concourse.bass as bass
    import concourse.bacc as bacc
    import concourse.mybir as mybir
    from concourse.bass_utils import run_bass_kernel_spmd
    import ml_dtypes
    bf = ml_dtypes.bfloat16

    x = np.asarray(x, np.float32)
    xt_all = np.ascontiguousarray(
        x.reshape(B * N, DIM).T.astype(bf))                  # [1024, 16640] bf16
    wqkvt = np.ascontiguousarray(np.asarray(w_qkv, np.float32).T.astype(bf))
    woutt_f32 = np.ascontiguousarray(np.asarray(w_out, np.float32).T)
    woutt = np.ascontiguousarray(woutt_f32.astype(bf))
    wc = np.asarray(conv_w, np.float32).transpose(2, 3, 1, 0).reshape(9, N, N)
    wconvt = np.zeros((9, N, 68), bf)
    wconvt[:, :, :N] = wc.astype(bf)
    s = np.asarray(bn_gamma, np.float32) / np.sqrt(np.asarray(bn_var, np.float32) + BN_EPS)
    t_aff = (np.asarray(conv_b, np.float32) - np.asarray(bn_mean, np.float32)) * s \
        + np.asarray(bn_beta, np.float32)
    st = np.ascontiguousarray(np.stack([s, np.zeros_like(s)], 1))
    bqkvc = np.ascontiguousarray(
        np.asarray(b_qkv, np.float32)[:2 * DIM].reshape(16, 128).T)
    identm = np.eye(128, dtype=bf)

    nc = bacc.Bacc()
    _build(nc, mybir, bass)
    nc.finalize()

    in_maps = []
    for c in range(NCORES):
        in_maps.append({
            "xt": np.ascontiguousarray(xt_all[:, c * TOK:(c + 1) * TOK]),
            "wqkvt": wqkvt, "woutt": woutt, "wconvt": wconvt, "st": st,
            "bqkvc": bqkvc, "ident": identm,
        })
    res = run_bass_kernel_spmd(nc, in_maps, list(range(NCORES)))
    outs = [res.results[c]["out"] for c in range(NCORES)]   # each [1024, 2080]
    full = np.concatenate(outs, axis=1).T.reshape(B, N, DIM)

    # exact host-side correction, batch-independent:
    #  - v/out biases: attn rows sum to 1 -> out1 += b_v; conv bias-image effect
    #  - BN additive term t_aff[n] (dropped in-kernel) contributes t_aff[n]
    #    at every feature of token n
    b_v = np.asarray(b_qkv, np.float32)[2 * DIM:]
    bimg = b_v.reshape(HEADS, DH)
    pad = np.zeros((HEADS + 2, DH + 2), np.float32)
    pad[1:-1, 1:-1] = bimg
    wsum = np.asarray(conv_w, np.float32).sum(1)      # [65, 3, 3]
    dconv = np.zeros((N, HEADS, DH), np.float32)
    for ty in range(3):
        for tx in range(3):
            dconv += wsum[:, ty, tx][:, None, None] * \
                pad[ty:ty + HEADS, tx:tx + DH][None, :, :]
    dres = b_v[None, :] + (dconv * s[:, None, None]).reshape(N, DIM) \
        + t_aff[:, None]
    # subtract the spurious dx=+1 ones-column contamination at x=63:
    # tap (ty, tx=2) adds wsum[n, ty] to conv[n, y, 63] for its valid y range
    corr = np.zeros((N, HEADS), np.float32)
    corr += wsum[:, 1, 2][:, None]                      # dy=0: all y
    corr[:, 1:] += wsum[:, 0, 2][:, None]               # dy=-1: y>=1
    corr[:, :HEADS - 1] += wsum[:, 2, 2][:, None]       # dy=+1: y<=14
    for y in range(HEADS):
        dres[:, y * DH + DH - 1] -= corr[:, y] * s
    dout = dres @ woutt_f32 + np.asarray(b_out, np.float32)[None, :]
    return full + dout[None, :, :]
